# revision 1
# baseline (speedup 1.0000x reference)
"""Trainium2 Bass kernel for a dense transformer block (nn_Block_50929722196345).

Problem: B=2, S=2048, D=1024, H=16 heads (hd=64), D_FF=4096, causal MHSA +
residual+LN1 + GELU FFN + residual+LN2 (flax-style, eps=1e-6).

Sharding across 8 NeuronCores (single SPMD program, all-static):
  - Token-sharded phases (QKV proj, out-proj, LNs, FFN): core c owns token
    chunk [256c, 256c+256) of BOTH batch elements (512 rows/core).
  - Head-sharded attention: core c owns heads {2c, 2c+1} of both batches
    (4 head-batches/core, full causal sequence) -- identical static causal
    loop structure on every core.
  - Per-batch AllToAlls (2+2) move Q^T/K^T/V_aug into head-sharding and
    attention outputs back, pipelined against compute.

Layout: activations transposed (features on partitions) for all matmuls.
V is augmented with a ones-column per head so the softmax denominator drops
out of the P@V matmul for free.  Matmuls run in float32r (full-rate fp32).
Softmax skips max-subtraction (scores provably small: |s*scale| < ~5).
"""

import numpy as np

import concourse.bass as bass
import concourse.mybir as mybir
import concourse.tile as tile
from concourse import bacc
from concourse.bass_utils import run_bass_kernel_spmd
from concourse.masks import make_identity

F32 = mybir.dt.float32
F32R = mybir.dt.float32r
AF = mybir.ActivationFunctionType
OP = mybir.AluOpType

NCORES = 8
B, S, D = 2, 2048, 1024
H, HD = 16, 64
DFF = 4096
SCALE = 1.0 / np.sqrt(HD)
EPS = 1e-6
TC = 256          # tokens per (core, batch)
TPC = 2 * TC      # tokens per core (both batches)
NDT = D // 128    # 8 feature tiles
NFT = DFF // 128  # 32 ff tiles
GROUPS = [[0, 1, 2, 3, 4, 5, 6, 7]]

QK_SHB = 128 * TC            # floats per Q (or K) per-batch A2A shard
V_SHB = TC * 130             # floats per V_aug per-batch shard
SHB = 2 * QK_SHB + V_SHB     # packed per-batch shard size

GELU_F = AF.Gelu_apprx_tanh  # CoreSim lacks tanh-approx; tests may override
_CACHED_NC = None


def _layernorm_T(nc, tc, src, dst, gamma, beta, ones_c128, ones_r128, eps_sb):
    """LayerNorm over features for feature-major (transposed) tiles.

    src/dst: [128, NDT, TPC]; gamma/beta: [128, NDT] per-partition params.
    Column statistics via ones-matmuls; mean/rstd broadcast via PE.
    """
    with tc.tile_pool(name="lnst", bufs=1) as lp, \
         tc.tile_pool(name="lnsq", bufs=2) as sqp, \
         tc.tile_pool(name="ps_st", bufs=1, space="PSUM") as ps_st, \
         tc.tile_pool(name="ps_lb", bufs=1, space="PSUM") as ps_lb:
        ps_sum = ps_st.tile([1, TPC], F32, name="ps_sum")
        ps_sq = ps_st.tile([1, TPC], F32, name="ps_sq")
        for dt in range(NDT):
            nc.tensor.matmul(ps_sum[:], ones_c128[:], src[:, dt, :],
                             start=(dt == 0), stop=(dt == NDT - 1))
        for dt in range(NDT):
            sq = sqp.tile([128, TPC], F32R, name="sq")
            nc.scalar.activation(sq[:], src[:, dt, :], AF.Square)
            nc.tensor.matmul(ps_sq[:], ones_c128[:], sq[:],
                             start=(dt == 0), stop=(dt == NDT - 1))
        m_sb = lp.tile([1, TPC], F32R, name="m_sb")
        nc.scalar.activation(m_sb[:], ps_sum[:], AF.Copy, scale=1.0 / D)
        e2_sb = lp.tile([1, TPC], F32, name="e2_sb")
        nc.scalar.activation(e2_sb[:], ps_sq[:], AF.Copy, scale=1.0 / D)
        msq = lp.tile([1, TPC], F32, name="msq")
        nc.vector.tensor_tensor(msq[:], m_sb[:], m_sb[:], op=OP.mult)
        var = lp.tile([1, TPC], F32, name="var")
        nc.vector.tensor_tensor(var[:], e2_sb[:], msq[:], op=OP.subtract)
        std = lp.tile([1, TPC], F32, name="std")
        nc.scalar.activation(std[:], var[:], AF.Sqrt, bias=eps_sb[:])
        rstd = lp.tile([1, TPC], F32R, name="rstd")
        with nc.allow_low_precision(reason="fp32r rounding of rstd is fine"):
            nc.vector.reciprocal(rstd[:], std[:])
        ps_m = ps_lb.tile([128, TPC], F32, name="ps_m")
        nc.tensor.matmul(ps_m[:], ones_r128[:], m_sb[:], start=True, stop=True)
        ps_r = ps_lb.tile([128, TPC], F32, name="ps_r")
        nc.tensor.matmul(ps_r[:], ones_r128[:], rstd[:], start=True, stop=True)
        rstd_bc = lp.tile([128, TPC], F32, name="rstd_bc")
        nc.vector.tensor_copy(rstd_bc[:], ps_r[:])
        for dt in range(NDT):
            t1 = sqp.tile([128, TPC], F32, name="t1")
            nc.vector.tensor_tensor(t1[:], src[:, dt, :], ps_m[:],
                                    op=OP.subtract)
            t2 = sqp.tile([128, TPC], F32, name="t2")
            nc.vector.tensor_tensor(t2[:], t1[:], rstd_bc[:], op=OP.mult)
            nc.vector.tensor_scalar(
                out=dst[:, dt, :], in0=t2[:],
                scalar1=gamma[:, dt:dt + 1], scalar2=beta[:, dt:dt + 1],
                op0=OP.mult, op1=OP.add)


def build_nc(sim_mode=False, phase_log=None, narrow=True, exp2bank=True, prefetch=True, nat_tail=True, use_ttr=False, niters=1, chain=False, merge_a2a=True):
    def mark(name):
        if phase_log is not None:
            phase_log.append((name, nc.next_id()))
    nc = bacc.Bacc("TRN2", target_bir_lowering=False, num_devices=NCORES)

    x_in = nc.dram_tensor("x_own", [TPC, D], F32, kind="ExternalInput")
    wq = nc.dram_tensor("wq", [D, D], F32, kind="ExternalInput")
    wk = nc.dram_tensor("wk", [D, D], F32, kind="ExternalInput")
    wv = nc.dram_tensor("wv", [D, D], F32, kind="ExternalInput")
    wo = nc.dram_tensor("wo", [D, D], F32, kind="ExternalInput")
    w1 = nc.dram_tensor("w1", [D, DFF], F32, kind="ExternalInput")
    w2 = nc.dram_tensor("w2", [DFF, D], F32, kind="ExternalInput")
    bq = nc.dram_tensor("bq", [D], F32, kind="ExternalInput")
    bk = nc.dram_tensor("bk", [D], F32, kind="ExternalInput")
    bv = nc.dram_tensor("bv", [D], F32, kind="ExternalInput")
    bo = nc.dram_tensor("bo", [D], F32, kind="ExternalInput")
    b1 = nc.dram_tensor("b1", [DFF], F32, kind="ExternalInput")
    b2 = nc.dram_tensor("b2", [D], F32, kind="ExternalInput")
    ln1_s = nc.dram_tensor("ln1_s", [D], F32, kind="ExternalInput")
    ln1_b = nc.dram_tensor("ln1_b", [D], F32, kind="ExternalInput")
    ln2_s = nc.dram_tensor("ln2_s", [D], F32, kind="ExternalInput")
    ln2_b = nc.dram_tensor("ln2_b", [D], F32, kind="ExternalInput")
    y_out = nc.dram_tensor("y", [TPC, D], F32, kind="ExternalOutput")

    def a2a(dst, srct, raw=False):
        if raw:
            if sim_mode:
                nc.sync.dma_start(dst, srct)
            else:
                nc.gpsimd.collective_compute(
                    "AllToAll", OP.bypass, replica_groups=GROUPS,
                    ins=[srct], outs=[dst])
            return
        if sim_mode:
            for d in range(NCORES):
                nc.sync.dma_start(dst[d], srct[d])
        else:
            nc.gpsimd.collective_compute(
                "AllToAll", OP.bypass, replica_groups=GROUPS,
                ins=[srct[:].opt()], outs=[dst[:].opt()])

    with tile.TileContext(nc) as tc:
        with tc.tile_pool(name="const", bufs=1) as cpool, \
             tc.tile_pool(name="dram", bufs=1, space="DRAM") as dr:

            if merge_a2a:
                a2a_in_m = dr.tile([NCORES, B, SHB], F32, name="a2a_in_m")
                a2a_out_m = dr.tile([NCORES, B, SHB], F32, name="a2a_out_m")
                a2o_in_m = dr.tile([NCORES, B, 128, TC], F32, name="a2o_in_m")
                a2o_out_m = dr.tile([NCORES, B, 128, TC], F32, name="a2o_out_m")
                a2a_in = [a2a_in_m[:, b, :] for b in range(B)]
                a2a_out = [a2a_out_m[:, b, :] for b in range(B)]
                a2o_in = [a2o_in_m[:, b, :, :] for b in range(B)]
                a2o_out = [a2o_out_m[:, b, :, :] for b in range(B)]
            else:
                a2a_in = [dr.tile([NCORES, SHB], F32, name=f"a2a_in{b}")
                          for b in range(B)]
                a2a_out = [dr.tile([NCORES, SHB], F32, name=f"a2a_out{b}")
                           for b in range(B)]
                a2o_in = [dr.tile([NCORES, 128, TC], F32, name=f"a2o_in{b}")
                          for b in range(B)]
                a2o_out = [dr.tile([NCORES, 128, TC], F32, name=f"a2o_out{b}")
                           for b in range(B)]
            chain_buf = dr.tile([128, D], F32, name="chain_buf")

            for _it in range(niters):
                # ========== P1: x load + transpose (emitted first: DMA priority)
                xT, xT_free = tc.tile([128, NDT, TPC], F32R, name="xT")
                ident = cpool.tile([128, 128], F32)
                make_identity(nc, ident[:])
                with tc.tile_pool(name="p1", bufs=2) as p1, \
                     tc.tile_pool(name="pst", bufs=4, space="PSUM") as pst:
                    for tt in range(TPC // 128):
                        x_nat = p1.tile([128, D], F32, name="x_nat")
                        nc.sync.dma_start(x_nat[:], x_in[128 * tt:128 * (tt + 1), :])
                        for dt in range(NDT):
                            ps_t = pst.tile([128, 128], F32, name="ps_t")
                            nc.tensor.transpose(
                                ps_t[:], x_nat[:, 128 * dt:128 * (dt + 1)], ident[:])
                            nc.vector.tensor_copy(
                                xT[:, dt, 128 * tt:128 * (tt + 1)], ps_t[:])

                mark("P1_xT")
                # ========== P2: QKV projections, per-batch halves ==========
                qt, qt_free = tc.tile([128, NDT, TPC], F32R, name="qt")
                ktl, kt_free = tc.tile([128, NDT, TPC], F32R, name="ktl")
                vaug, vaug_free = tc.tile([128, TPC // 128, H, 65], F32R, name="vaug")
                wq_sb, wq_free = tc.tile([128, NDT, D], F32R, name="wq_sb")
                wk_sb, wk_free = tc.tile([128, NDT, D], F32R, name="wk_sb")
                wv_sb, wv_free = tc.tile([128, NDT, D], F32R, name="wv_sb")
                for w_sb, w_dram in ((wq_sb, wq), (wk_sb, wk), (wv_sb, wv)):
                    for ct in range(NDT):
                        nc.sync.dma_start(
                            w_sb[:, ct, :],
                            w_dram[128 * ct:128 * (ct + 1), :].rearrange(
                                "(o p) d -> p o d", p=128)[:, 0, :].bitcast(F32R))

                # constants & per-partition params (after big DMAs in queue order)
                ones_f32 = cpool.tile([128, 128], F32)
                nc.vector.memset(ones_f32[:], 1.0)
                ones_c128 = cpool.tile([128, 1], F32R)
                nc.vector.tensor_copy(ones_c128[:], ones_f32[:, 0:1])
                ones_r128 = cpool.tile([1, 128], F32R)
                nc.vector.tensor_copy(ones_r128[:], ones_f32[0:1, :])
                # sliding causal mask: M[p, u] = 1 iff u - p >= 512
                # diag k-tile (relative index r in 0..3 within a 512-q window)
                # uses slice M[:, 512-128r : 1024-128r]
                mask_f32 = cpool.tile([128, 1024], F32)
                nc.gpsimd.memset(mask_f32[:], 1.0)
                nc.gpsimd.affine_select(
                    out=mask_f32[:], in_=mask_f32[:],
                    compare_op=OP.is_ge, fill=0.0, base=-512,
                    pattern=[[1, 1024]], channel_multiplier=-1,
                )
                diag_mask = cpool.tile([128, 1024], F32R)
                nc.vector.tensor_copy(diag_mask[:], mask_f32[:])

                def load_pp(name, t, n):
                    sb = cpool.tile([128, n], F32, name=name)
                    nc.sync.dma_start(sb[:], t[:].rearrange("(a p) -> p a", p=128))
                    return sb

                bq_sb = load_pp("bq_sb", bq, NDT)
                bk_sb = load_pp("bk_sb", bk, NDT)
                bo_sb = load_pp("bo_sb", bo, NDT)
                b1_sb = load_pp("b1_sb", b1, NFT)
                g1_sb = load_pp("g1_sb", ln1_s, NDT)
                be1_sb = load_pp("be1_sb", ln1_b, NDT)

                def load_bc(name, t):
                    sb = cpool.tile([128, D], F32, name=name)
                    nc.sync.dma_start(
                        sb[:], t[:].rearrange("(o d) -> o d", o=1)
                            .partition_broadcast(128)[:, 0, :])
                    return sb

                b2_bc = load_bc("b2_bc", b2)
                g2_bc = load_bc("g2_bc", ln2_s)
                be2_bc = load_bc("be2_bc", ln2_b)
                if not nat_tail:
                    b2_sb = load_pp("b2_sb", b2, NDT)
                    g2_sb = load_pp("g2_sb", ln2_s, NDT)
                    be2_sb = load_pp("be2_sb", ln2_b, NDT)
                eps_sb = cpool.tile([1, 1], F32)
                nc.vector.memset(eps_sb[:], float(EPS))
                eps_sb_p = cpool.tile([128, 1], F32)
                nc.vector.memset(eps_sb_p[:], float(EPS))
                bv_bc = cpool.tile([128, D], F32)
                nc.sync.dma_start(
                    bv_bc[:],
                    bv[:].rearrange("(o d) -> o d", o=1).partition_broadcast(128)[:, 0, :])

                with tc.tile_pool(name="psA", bufs=2, space="PSUM") as psA:
                    for beta in range(B):
                        c0 = TC * beta
                        for dt in range(NDT):
                            ps_q = psA.tile([128, TC], F32, name="ps_q")
                            for ct in range(NDT):
                                nc.tensor.matmul(
                                    ps_q[:], wq_sb[:, ct, 128 * dt:128 * (dt + 1)],
                                    xT[:, ct, c0:c0 + TC],
                                    start=(ct == 0), stop=(ct == NDT - 1))
                            nc.vector.tensor_scalar(
                                out=qt[:, dt, c0:c0 + TC], in0=ps_q[:],
                                scalar1=bq_sb[:, dt:dt + 1], scalar2=None, op0=OP.add)
                        for dt in range(NDT):
                            ps_k = psA.tile([128, TC], F32, name="ps_k")
                            for ct in range(NDT):
                                nc.tensor.matmul(
                                    ps_k[:], wk_sb[:, ct, 128 * dt:128 * (dt + 1)],
                                    xT[:, ct, c0:c0 + TC],
                                    start=(ct == 0), stop=(ct == NDT - 1))
                            nc.vector.tensor_scalar(
                                out=ktl[:, dt, c0:c0 + TC], in0=ps_k[:],
                                scalar1=bk_sb[:, dt:dt + 1], scalar2=None, op0=OP.add)
                        for tt in range(2 * beta, 2 * beta + 2):
                            for hf in range(2):
                                ps_v = psA.tile([128, 512], F32, name="ps_v")
                                for ct in range(NDT):
                                    nc.tensor.matmul(
                                        ps_v[:], xT[:, ct, 128 * tt:128 * (tt + 1)],
                                        wv_sb[:, ct, 512 * hf:512 * (hf + 1)],
                                        start=(ct == 0), stop=(ct == NDT - 1))
                                nc.vector.scalar_tensor_tensor(
                                    out=vaug[:, tt, 8 * hf:8 * (hf + 1), 0:64],
                                    in0=ps_v[:].rearrange("p (h e) -> p h e", h=8),
                                    scalar=1.0,
                                    in1=bv_bc[:, 512 * hf:512 * (hf + 1)].rearrange(
                                        "p (h e) -> p h e", h=8),
                                    op0=OP.mult, op1=OP.add)
                            nc.vector.tensor_copy(vaug[:, tt, :, 64:65],
                                                  ones_f32[:, 0:16, None])

                        # pack + A2A for this batch:
                        # shard d = (Q dims dt=d | K dims dt=d | V heads {2d,2d+1})
                        for d in range(NCORES):
                            nc.sync.dma_start(
                                a2a_in[beta][d, 0:QK_SHB]
                                    .rearrange("(p t) -> p t", p=128),
                                qt[:, d, c0:c0 + TC].bitcast(F32))
                            nc.sync.dma_start(
                                a2a_in[beta][d, QK_SHB:2 * QK_SHB]
                                    .rearrange("(p t) -> p t", p=128),
                                ktl[:, d, c0:c0 + TC].bitcast(F32))
                            nc.sync.dma_start(
                                a2a_in[beta][d, 2 * QK_SHB:SHB]
                                    .rearrange("(t c) -> t c", c=130)
                                    .rearrange("(tt p) c -> p tt c", p=128),
                                vaug[:, 2 * beta:2 * beta + 2, 2 * d:2 * d + 2, :]
                                    .rearrange("p tt h c -> p tt (h c)").bitcast(F32))
                        if merge_a2a:
                            if beta == B - 1:
                                a2a(a2a_out_m[:].opt(), a2a_in_m[:].opt(),
                                    raw=True)
                        else:
                            a2a(a2a_out[beta], a2a_in[beta])
                        mark(f"P2_qkv_b{beta}")

                wv_free()
                wk_free()
                wq_free()
                vaug_free()
                kt_free()
                qt_free()

                r1, r1_free = tc.tile([128, NDT, TPC], F32R, name="r1")
                # preload wo for P4 while attention runs
                wo_sb, wo_free = tc.tile([128, NDT, D], F32R, name="wo_sb")
                for ct in range(NDT):
                    nc.sync.dma_start(
                        wo_sb[:, ct, :],
                        wo[128 * ct:128 * (ct + 1), :].rearrange(
                            "(o p) d -> p o d", p=128)[:, 0, :].bitcast(F32R))

                # ========== P3: attention (my 2 heads x 2 batches) ==========
                NW = S // 512  # 4 q-windows of 512
                with tc.tile_pool(name="att_io", bufs=2) as aio, \
                     tc.tile_pool(name="exp", bufs=3) as epool, \
                     tc.tile_pool(name="stage", bufs=4) as spool, \
                     tc.tile_pool(name="ps_sc", bufs=2, space="PSUM") as ps_sc, \
                     tc.tile_pool(name="ps_pv", bufs=2, space="PSUM") as ps_pv, \
                     tc.tile_pool(name="ps_bc", bufs=2, space="PSUM") as ps_bc:

                    for beta in range(B):
                        q_sb = aio.tile([128, S], F32R, name="q_sb")
                        k_sb = aio.tile([128, S], F32R, name="k_sb")
                        va_sb = aio.tile([128, S // 128, 130], F32R, name="va_sb")
                        for s in range(NCORES):
                            nc.sync.dma_start(
                                q_sb[:, TC * s:TC * (s + 1)],
                                a2a_out[beta][s, 0:QK_SHB]
                                    .rearrange("(p t) -> p t", p=128).bitcast(F32R))
                            nc.sync.dma_start(
                                k_sb[:, TC * s:TC * (s + 1)],
                                a2a_out[beta][s, QK_SHB:2 * QK_SHB]
                                    .rearrange("(p t) -> p t", p=128).bitcast(F32R))
                            nc.sync.dma_start(
                                va_sb[:, 2 * s:2 * s + 2, :],
                                a2a_out[beta][s, 2 * QK_SHB:SHB]
                                    .rearrange("(t c) -> t c", c=130)
                                    .rearrange("(u p) c -> p u c", p=128)
                                    .bitcast(F32R))

                        for w in range(NW):
                            q0 = 512 * w
                            for j in range(2):  # my two heads
                                r0 = 64 * j
                                ps_o = ps_pv.tile([65, 512], F32, name="ps_o")
                                npair = 2 * w + 2
                                for pr in range(npair):
                                    if exp2bank:
                                        ps_s = ps_sc.tile([128, 1024], F32,
                                                          name="ps_s")
                                        pss = [ps_s[:, 0:512], ps_s[:, 512:1024]]
                                    else:
                                        a_ = ps_sc.tile([128, 512], F32,
                                                        name="ps_sa", tag="ps_sa")
                                        b_ = ps_sc.tile([128, 512], F32,
                                                        name="ps_sb", tag="ps_sb")
                                        pss = [a_[:], b_[:]]
                                    ex = epool.tile([128, 1024], F32R, name="ex")
                                    rels = [2 * pr - 4 * w, 2 * pr + 1 - 4 * w]
                                    if not narrow:
                                        rels = [min(r, 0) for r in rels]
                                    relm = [2 * pr - 4 * w, 2 * pr + 1 - 4 * w]
                                    for u in range(2):
                                        kt_i = 2 * pr + u
                                        qlo = max(0, 128 * rels[u])
                                        nc.tensor.matmul(
                                            pss[u][:, qlo:512],
                                            k_sb[r0:r0 + 64,
                                                 128 * kt_i:128 * (kt_i + 1)],
                                            q_sb[r0:r0 + 64, q0 + qlo:q0 + 512],
                                            start=True, stop=True)
                                    if exp2bank and relm[0] < 0 and relm[1] < 0:
                                        # both tiles fully visible: one wide exp
                                        nc.scalar.activation(ex[:], ps_s[:], AF.Exp,
                                                             scale=float(SCALE))
                                    else:
                                        for u in range(2):
                                            qlo = max(0, 128 * rels[u])
                                            nc.scalar.activation(
                                                ex[:, 512 * u + qlo:512 * (u + 1)],
                                                pss[u][:, qlo:512],
                                                AF.Exp, scale=float(SCALE))
                                    for u in range(2):
                                        kt_i = 2 * pr + u
                                        qlo = max(0, 128 * rels[u])
                                        if relm[u] >= 0:
                                            # triangle mask on the narrowed range
                                            moff = 512 - 128 * (relm[u] - rels[u] if rels[u] > 0 else relm[u] - 0)
                                            moff = 512 - 128 * (relm[u] if qlo == 0 else 0)
                                            nc.vector.tensor_tensor(
                                                ex[:, 512 * u + qlo:512 * (u + 1)],
                                                ex[:, 512 * u + qlo:512 * (u + 1)],
                                                diag_mask[:, moff:moff + 512 - qlo],
                                                op=OP.mult)
                                        nc.tensor.matmul(
                                            ps_o[:, qlo:512],
                                            va_sb[:, kt_i, 65 * j:65 * (j + 1)],
                                            ex[:, 512 * u + qlo:512 * (u + 1)],
                                            start=(kt_i == 0),
                                            stop=(kt_i == 4 * w + 3))
                                # normalize by ones-row denominator
                                recip = spool.tile([1, 512], F32R, name="recip")
                                with nc.allow_low_precision(
                                        reason="fp32r rounding of softmax denom"):
                                    nc.vector.reciprocal(recip[:], ps_o[64:65, :])
                                ps_b = ps_bc.tile([64, 512], F32, name="ps_b")
                                nc.tensor.matmul(ps_b[:], ones_r128[:, 0:64],
                                                 recip[:], start=True, stop=True)
                                rb = spool.tile([64, 512], F32, name="rb")
                                nc.vector.tensor_copy(rb[:], ps_b[:])
                                stg = spool.tile([64, 512], F32, name="stg")
                                nc.vector.tensor_tensor(
                                    stg[:], ps_o[0:64, :], rb[:], op=OP.mult)
                                for h in range(2):  # two dest token chunks
                                    nc.sync.dma_start(
                                        a2o_in[beta][2 * w + h, r0:r0 + 64, :],
                                        stg[:, 256 * h:256 * (h + 1)])
                        if merge_a2a:
                            if beta == B - 1:
                                a2a(a2o_out_m[:].opt(), a2o_in_m[:].opt(),
                                    raw=True)
                        else:
                            a2a(a2o_out[beta], a2o_in[beta])
                        mark(f"P3_attn_b{beta}")

                # assemble attn_T and out-proj, per batch halves
                attn_sb, attn_free = tc.tile([128, NDT, TPC], F32R, name="attn_sb")
                with tc.tile_pool(name="psB", bufs=4, space="PSUM") as psB:
                    for beta in range(B):
                        c0 = TC * beta
                        for s in range(NCORES):
                            nc.sync.dma_start(attn_sb[:, s, c0:c0 + TC],
                                              a2o_out[beta][s].bitcast(F32R))
                        for dt in range(NDT):
                            ps_po = psB.tile([128, TC], F32, name="ps_po")
                            for ct in range(NDT):
                                nc.tensor.matmul(
                                    ps_po[:], wo_sb[:, ct, 128 * dt:128 * (dt + 1)],
                                    attn_sb[:, ct, c0:c0 + TC],
                                    start=(ct == 0), stop=(ct == NDT - 1))
                            nc.vector.scalar_tensor_tensor(
                                out=r1[:, dt, c0:c0 + TC], in0=ps_po[:],
                                scalar=bo_sb[:, dt:dt + 1], in1=xT[:, dt, c0:c0 + TC],
                                op0=OP.add, op1=OP.add)
                mark("P4_oproj")
                attn_free()
                wo_free()

                # ========== P5: LN1 ==========
                ln1, ln1_free = tc.tile([128, NDT, TPC], F32R, name="ln1", side="right")
                _layernorm_T(nc, tc, r1, ln1, g1_sb, be1_sb, ones_c128, ones_r128,
                             eps_sb)
                mark("P5_ln1")
                r1_free()
                xT_free()

                # ln1 -> natural layout (+b2 folded) for the FFN2 residual path
                ln1nb, ln1nb_free = (None, (lambda: None))
                if nat_tail:
                    ln1nb, ln1nb_free = tc.tile([128, TPC // 128, D], F32, name="ln1nb")
                with tc.tile_pool(name="pstn", bufs=4, space="PSUM") as pstn:
                  if nat_tail:
                    for tt in range(TPC // 128):
                        for dt in range(NDT):
                            ps_tn = pstn.tile([128, 128], F32, name="ps_tn")
                            nc.tensor.transpose(
                                ps_tn[:],
                                ln1[:, dt, 128 * tt:128 * (tt + 1)].bitcast(F32),
                                ident[:])
                            nc.vector.tensor_tensor(
                                ln1nb[:, tt, 128 * dt:128 * (dt + 1)],
                                ps_tn[:], b2_bc[:, 128 * dt:128 * (dt + 1)],
                                op=OP.add)
                  else:
                    pass

                # ========== P6/P7: FFN (w1/w2 streamed via right-side pools) ==
                with tc.tile_pool(name="w1s", bufs=2, side="right") as w1pool, \
                     tc.tile_pool(name="w2s", bufs=2, side="right") as w2pool:
                    gT, gT_free = tc.tile([128, NFT, TPC], F32R, name="gT",
                                          side="right")
                    def _w1dma(fb):
                        w1_sb = w1pool.tile([128, NDT, 512], F32R, name="w1_sb")
                        nc.sync.dma_start(
                            w1_sb[:],
                            w1[:, 512 * fb:512 * (fb + 1)]
                                .rearrange("(c p) f -> p c f", p=128).bitcast(F32R))
                        return w1_sb

                    def _w2dma(ftb):
                        w2_sb = w2pool.tile([128, 4, D], F32R, name="w2_sb")
                        nc.sync.dma_start(
                            w2_sb[:],
                            w2[512 * ftb:512 * (ftb + 1), :]
                                .rearrange("(f p) d -> p f d", p=128).bitcast(F32R))
                        return w2_sb

                    w1_tiles = {}
                    w2_tiles = {}
                    if prefetch:
                        for fb in range(NFT // 4):
                            w1_tiles[fb] = _w1dma(fb)
                        for ftb in range(NFT // 4):
                            w2_tiles[ftb] = _w2dma(ftb)

                    with tc.tile_pool(name="psC", bufs=2, space="PSUM") as psC:
                        for fb in range(NFT // 4):
                            w1_sb = w1_tiles[fb] if prefetch else _w1dma(fb)
                            for fc in range(4):
                                ft = 4 * fb + fc
                                ps_h = psC.tile([128, TPC], F32, name="ps_h")
                                for ct in range(NDT):
                                    nc.tensor.matmul(
                                        ps_h[:],
                                        w1_sb[:, ct, 128 * fc:128 * (fc + 1)],
                                        ln1[:, ct, :],
                                        start=(ct == 0), stop=(ct == NDT - 1))
                                nc.scalar.activation(gT[:, ft, :], ps_h[:],
                                                     GELU_F,
                                                     bias=b1_sb[:, ft:ft + 1])
                    mark("P6_ffn1")

                    if not nat_tail:
                        r2, r2_free = tc.tile([128, NDT, TPC], F32R, name="r2")
                        with tc.tile_pool(name="psDo", bufs=1, space="PSUM") as psDo:
                            ps_yo = [psDo.tile([128, TPC], F32, name=f"ps_yo{dc}")
                                     for dc in range(NDT)]
                            for ftb in range(NFT // 4):
                                w2_sb = w2_tiles[ftb] if prefetch else _w2dma(ftb)
                                for fl in range(4):
                                    for dc in range(NDT):
                                        nc.tensor.matmul(
                                            ps_yo[dc][:],
                                            w2_sb[:, fl, 128 * dc:128 * (dc + 1)],
                                            gT[:, 4 * ftb + fl, :],
                                            start=(ftb == 0 and fl == 0),
                                            stop=(ftb == NFT // 4 - 1 and fl == 3))
                            for dc in range(NDT):
                                nc.vector.scalar_tensor_tensor(
                                    out=r2[:, dc, :], in0=ps_yo[dc][:],
                                    scalar=b2_sb[:, dc:dc + 1], in1=ln1[:, dc, :],
                                    op0=OP.add, op1=OP.add)
                        gT_free()
                        ln2, ln2_free = tc.tile([128, NDT, TPC], F32, name="ln2")
                        _layernorm_T(nc, tc, r2, ln2, g2_sb, be2_sb, ones_c128,
                                     ones_r128, eps_sb)
                        with tc.tile_pool(name="outp", bufs=2) as opool, \
                             tc.tile_pool(name="psE", bufs=4, space="PSUM") as psE:
                            for tt in range(TPC // 128):
                                o_sb = opool.tile([128, D], F32, name="o_sb")
                                for dt in range(NDT):
                                    ps_t2 = psE.tile([128, 128], F32, name="ps_t2")
                                    nc.tensor.transpose(
                                        ps_t2[:],
                                        ln2[:, dt, 128 * tt:128 * (tt + 1)],
                                        ident[:])
                                    nc.vector.tensor_copy(
                                        o_sb[:, 128 * dt:128 * (dt + 1)], ps_t2[:])
                                nc.sync.dma_start(
                                    y_out[128 * tt:128 * (tt + 1), :], o_sb[:])
                        ln2_free()
                        r2_free()
                    elif True:
                        with tc.tile_pool(name="psD", bufs=1, space="PSUM") as psD:
                          ps_y = [psD.tile([128, 512], F32, name=f"ps_y{i}")
                                  for i in range(8)]
                          for ftb in range(NFT // 4):
                              w2_sb = w2_tiles[ftb] if prefetch else _w2dma(ftb)
                              for fl in range(4):
                                  ft = 4 * ftb + fl
                                  for tt in range(TPC // 128):
                                      for dh in range(2):
                                          nc.tensor.matmul(
                                              ps_y[2 * tt + dh][:],
                                              gT[:, ft, 128 * tt:128 * (tt + 1)],
                                              w2_sb[:, fl, 512 * dh:512 * (dh + 1)],
                                              start=(ft == 0),
                                              stop=(ft == NFT - 1))
                          gT_free()
                          # residual + LN2 (natural, per token tile) + store
                          with tc.tile_pool(name="lnn", bufs=2) as lnn, \
                               tc.tile_pool(name="lnsc", bufs=2) as lnsc:
                              for tt in range(TPC // 128):
                                  r2n = lnn.tile([128, D], F32, name="r2n")
                                  for dh in range(2):
                                      nc.vector.tensor_tensor(
                                          r2n[:, 512 * dh:512 * (dh + 1)],
                                          ps_y[2 * tt + dh][:],
                                          ln1nb[:, tt, 512 * dh:512 * (dh + 1)],
                                          op=OP.add)
                                  ssum = lnsc.tile([128, 1], F32, name="ssum")
                                  nc.vector.tensor_reduce(ssum[:], r2n[:],
                                                          axis=mybir.AxisListType.X,
                                                          op=OP.add)
                                  sqs = lnsc.tile([128, D], F32, name="sqs")
                                  s2 = lnsc.tile([128, 1], F32, name="s2")
                                  if use_ttr:
                                      nc.vector.tensor_tensor_reduce(
                                          out=sqs[:], in0=r2n[:], in1=r2n[:],
                                          scale=1.0, scalar=0.0, op0=OP.mult,
                                          op1=OP.add, accum_out=s2[:])
                                  else:
                                      nc.scalar.activation(sqs[:], r2n[:],
                                                           AF.Square)
                                      nc.vector.tensor_reduce(
                                          s2[:], sqs[:],
                                          axis=mybir.AxisListType.X, op=OP.add)
                                  m = lnsc.tile([128, 1], F32, name="m")
                                  nc.vector.tensor_scalar(out=m[:], in0=ssum[:],
                                                          scalar1=1.0 / D,
                                                          scalar2=None, op0=OP.mult)
                                  e2 = lnsc.tile([128, 1], F32, name="e2")
                                  nc.vector.tensor_scalar(out=e2[:], in0=s2[:],
                                                          scalar1=1.0 / D,
                                                          scalar2=None, op0=OP.mult)
                                  msq = lnsc.tile([128, 1], F32, name="msq")
                                  nc.vector.tensor_tensor(msq[:], m[:], m[:],
                                                          op=OP.mult)
                                  var = lnsc.tile([128, 1], F32, name="var")
                                  nc.vector.tensor_tensor(var[:], e2[:], msq[:],
                                                          op=OP.subtract)
                                  std = lnsc.tile([128, 1], F32, name="std")
                                  nc.scalar.activation(std[:], var[:], AF.Sqrt,
                                                       bias=eps_sb_p[:])
                                  rstd = lnsc.tile([128, 1], F32, name="rstd")
                                  nc.vector.reciprocal(rstd[:], std[:])
                                  t_n = lnsc.tile([128, D], F32, name="t_n")
                                  nc.vector.tensor_scalar(
                                      out=t_n[:], in0=r2n[:], scalar1=m[:],
                                      scalar2=rstd[:], op0=OP.subtract, op1=OP.mult)
                                  t_g = lnsc.tile([128, D], F32, name="t_g")
                                  nc.vector.tensor_tensor(t_g[:], t_n[:], g2_bc[:],
                                                          op=OP.mult)
                                  o_n = lnn.tile([128, D], F32, name="o_n")
                                  nc.vector.tensor_tensor(o_n[:], t_g[:], be2_bc[:],
                                                          op=OP.add)
                                  nc.sync.dma_start(
                                      y_out[128 * tt:128 * (tt + 1), :], o_n[:])
                mark("P7_ffn2")
                ln1_free()
                ln1nb_free()
                mark("P9_out")

    nc.finalize()
    return nc


def _get_nc():
    global _CACHED_NC
    if _CACHED_NC is None:
        _CACHED_NC = build_nc()
    return _CACHED_NC


def kernel(x, attention_mask, wq, bq, wk, bk, wv, bv, wo, bo,
           ln1_scale, ln1_bias, w1, b1, w2, b2, ln2_scale, ln2_bias):
    x = np.ascontiguousarray(np.asarray(x, dtype=np.float32))
    shared = {
        "wq": np.ascontiguousarray(np.asarray(wq, np.float32)),
        "wk": np.ascontiguousarray(np.asarray(wk, np.float32)),
        "wv": np.ascontiguousarray(np.asarray(wv, np.float32)),
        "wo": np.ascontiguousarray(np.asarray(wo, np.float32)),
        "w1": np.ascontiguousarray(np.asarray(w1, np.float32)),
        "w2": np.ascontiguousarray(np.asarray(w2, np.float32)),
        "bq": np.asarray(bq, np.float32), "bk": np.asarray(bk, np.float32),
        "bv": np.asarray(bv, np.float32), "bo": np.asarray(bo, np.float32),
        "b1": np.asarray(b1, np.float32), "b2": np.asarray(b2, np.float32),
        "ln1_s": np.asarray(ln1_scale, np.float32),
        "ln1_b": np.asarray(ln1_bias, np.float32),
        "ln2_s": np.asarray(ln2_scale, np.float32),
        "ln2_b": np.asarray(ln2_bias, np.float32),
    }
    in_maps = []
    for c in range(NCORES):
        x_own = np.concatenate(
            [x[0, TC * c:TC * (c + 1)], x[1, TC * c:TC * (c + 1)]], axis=0)
        in_maps.append({"x_own": np.ascontiguousarray(x_own), **shared})

    nc = _get_nc()
    res = run_bass_kernel_spmd(nc, in_maps, core_ids=list(range(NCORES)))
    out = np.empty((B, S, D), np.float32)
    for c in range(NCORES):
        y = res.results[c]["y"]
        out[0, TC * c:TC * (c + 1)] = y[0:TC]
        out[1, TC * c:TC * (c + 1)] = y[TC:TPC]
    return out



# revision 23
# speedup vs baseline: 997.4536x; 997.4536x over previous
"""Trainium2 Bass kernel for a dense transformer block (nn_Block_50929722196345).

Problem: B=2, S=2048, D=1024, H=16 heads (hd=64), D_FF=4096, causal MHSA +
residual+LN1 + GELU FFN + residual+LN2 (flax-style, eps=1e-6).

Sharding across 8 NeuronCores (single SPMD program, all-static):
  - Token-sharded phases (QKV proj, out-proj, LNs, FFN): core c owns token
    chunk [256c, 256c+256) of BOTH batch elements (512 rows/core).
  - Head-sharded attention: core c owns heads {2c, 2c+1} of both batches.
  - Per-batch AllToAlls move Q/K/V into head-sharding and attention outputs
    back (fp8 payloads).

Precision strategy (validated against the jax reference in numpy emulation):
  - QKV, scores, P@V, out-proj matmuls run in fp8e4m3 with DoubleRow perf
    mode (2 contraction elements per partition -> 0.5 cycles/row on PE).
    Weights are pre-scaled x32, packed host-side into [p, pair] layout;
    activations pair across adjacent 128-feature tiles so every DoubleRow
    operand is a native strided view.
  - FFN1/FFN2 run in bf16 (these dominate the error budget in fp8).
  - Residuals, layernorms, softmax denominators stay fp32.
  - exp uses bias -2.5 so fp8 attention weights stay in range; the constant
    cancels in the softmax ratio.
"""

import numpy as np
import ml_dtypes

import concourse.bass as bass
import concourse.mybir as mybir
import concourse.tile as tile
from concourse import bacc
from concourse.bass_utils import run_bass_kernel_spmd
from concourse.masks import make_identity

F32 = mybir.dt.float32
F32R = mybir.dt.float32r
F8 = mybir.dt.float8e4
BF = mybir.dt.bfloat16
AF = mybir.ActivationFunctionType
OP = mybir.AluOpType
DR = mybir.MatmulPerfMode.DoubleRow

NCORES = 8
B, S, D = 2, 2048, 1024
H, HD = 16, 64
DFF = 4096
SCALE = 1.0 / np.sqrt(HD)
EPS = 1e-6
EXB = -2.5          # exp bias, cancels in softmax ratio
WS = 32.0           # fp8 weight scale
IWS = 1.0 / WS
TC = 256            # tokens per (core, batch)
TPC = 2 * TC        # tokens per core (both batches)
NDT = D // 128      # 8 feature tiles
NFT = DFF // 128    # 32 ff tiles
GROUPS = [[0, 1, 2, 3, 4, 5, 6, 7]]

QK_SHB = 128 * TC            # bytes per Q (or K) per-batch A2A shard (fp8)
V_SHB = 2 * 2 * 64 * 128     # bytes per V_aug per-batch shard (fp8)
SHB = 2 * QK_SHB + V_SHB     # packed per-batch shard size (bytes, fp8)

GELU_F = AF.Gelu_apprx_tanh
_CACHED_NC = None


def _layernorm_T(nc, tc, src, dst, gamma, beta, ones_c128, ones_r128, eps_sb):
    """LayerNorm over features for feature-major (transposed) tiles.

    src/dst: [128, NDT, TPC]; gamma/beta: [128, NDT] per-partition params.
    Column statistics via ones-matmuls; mean/rstd broadcast via PE.
    """
    with tc.tile_pool(name="lnst", bufs=1) as lp, \
         tc.tile_pool(name="lnsq", bufs=2) as sqp, \
         tc.tile_pool(name="ps_st", bufs=1, space="PSUM") as ps_st, \
         tc.tile_pool(name="ps_lb", bufs=1, space="PSUM") as ps_lb:
        ps_sum = ps_st.tile([1, TPC], F32, name="ps_sum")
        ps_sq = ps_st.tile([1, TPC], F32, name="ps_sq")
        for dt in range(NDT):
            nc.tensor.matmul(ps_sum[:], ones_c128[:], src[:, dt, :],
                             start=(dt == 0), stop=(dt == NDT - 1))
        for dt in range(NDT):
            sq = sqp.tile([128, TPC], F32R, name="sq")
            nc.scalar.activation(sq[:], src[:, dt, :], AF.Square)
            nc.tensor.matmul(ps_sq[:], ones_c128[:], sq[:],
                             start=(dt == 0), stop=(dt == NDT - 1))
        m_sb = lp.tile([1, TPC], F32R, name="m_sb")
        nc.scalar.activation(m_sb[:], ps_sum[:], AF.Copy, scale=1.0 / D)
        e2_sb = lp.tile([1, TPC], F32, name="e2_sb")
        nc.scalar.activation(e2_sb[:], ps_sq[:], AF.Copy, scale=1.0 / D)
        msq = lp.tile([1, TPC], F32, name="msq")
        nc.vector.tensor_tensor(msq[:], m_sb[:], m_sb[:], op=OP.mult)
        var = lp.tile([1, TPC], F32, name="var")
        nc.vector.tensor_tensor(var[:], e2_sb[:], msq[:], op=OP.subtract)
        std = lp.tile([1, TPC], F32, name="std")
        nc.scalar.activation(std[:], var[:], AF.Sqrt, bias=eps_sb[:])
        rstd = lp.tile([1, TPC], F32R, name="rstd")
        with nc.allow_low_precision(reason="fp32r rounding of rstd is fine"):
            nc.vector.reciprocal(rstd[:], std[:])
        ps_m = ps_lb.tile([128, TPC], F32, name="ps_m")
        nc.tensor.matmul(ps_m[:], ones_r128[:], m_sb[:], start=True, stop=True)
        ps_r = ps_lb.tile([128, TPC], F32, name="ps_r")
        nc.tensor.matmul(ps_r[:], ones_r128[:], rstd[:], start=True, stop=True)
        rstd_bc = lp.tile([128, TPC], F32, name="rstd_bc")
        nc.vector.tensor_copy(rstd_bc[:], ps_r[:])
        for dt in range(NDT):
            t1 = sqp.tile([128, TPC], F32, name="t1")
            nc.vector.tensor_tensor(t1[:], src[:, dt, :], ps_m[:],
                                    op=OP.subtract)
            t2 = sqp.tile([128, TPC], F32, name="t2")
            nc.vector.tensor_tensor(t2[:], t1[:], rstd_bc[:], op=OP.mult)
            nc.vector.tensor_scalar(
                out=dst[:, dt, :], in0=t2[:],
                scalar1=gamma[:, dt:dt + 1], scalar2=beta[:, dt:dt + 1],
                op0=OP.mult, op1=OP.add)


def build_nc(sim_mode=False, phase_log=None, niters=1, merge_a2a=True):
    def mark(name):
        if phase_log is not None:
            phase_log.append((name, nc.next_id()))
    nc = bacc.Bacc("TRN2", target_bir_lowering=False, num_devices=NCORES)

    x_in = nc.dram_tensor("x_own", [TPC, D], F32, kind="ExternalInput")
    # fp8 DoubleRow-packed weights: [128, m(4), i(2), col] * WS
    wq8 = nc.dram_tensor("wq8", [128, 8192], F8, kind="ExternalInput")
    wk8 = nc.dram_tensor("wk8", [128, 8192], F8, kind="ExternalInput")
    wv8 = nc.dram_tensor("wv8", [128, 8192], F8, kind="ExternalInput")
    wo8 = nc.dram_tensor("wo8", [128, 8192], F8, kind="ExternalInput")
    w1h = nc.dram_tensor("w1h", [D, DFF], BF, kind="ExternalInput")
    w2h = nc.dram_tensor("w2h", [DFF, D], BF, kind="ExternalInput")
    bq = nc.dram_tensor("bq", [D], F32, kind="ExternalInput")
    bk = nc.dram_tensor("bk", [D], F32, kind="ExternalInput")
    bv = nc.dram_tensor("bv", [D], F32, kind="ExternalInput")
    bo = nc.dram_tensor("bo", [D], F32, kind="ExternalInput")
    b1 = nc.dram_tensor("b1", [DFF], F32, kind="ExternalInput")
    b2 = nc.dram_tensor("b2", [D], F32, kind="ExternalInput")
    ln1_s = nc.dram_tensor("ln1_s", [D], F32, kind="ExternalInput")
    ln1_b = nc.dram_tensor("ln1_b", [D], F32, kind="ExternalInput")
    ln2_s = nc.dram_tensor("ln2_s", [D], F32, kind="ExternalInput")
    ln2_b = nc.dram_tensor("ln2_b", [D], F32, kind="ExternalInput")
    y_out = nc.dram_tensor("y", [TPC, D], F32, kind="ExternalOutput")

    def a2a(dst, srct):
        if sim_mode:
            nc.sync.dma_start(dst, srct)
        else:
            # bitcast the fp8 payload to f32 for the collective: 1-byte
            # dtypes hit a pathologically slow AllToAll path on HW.
            nc.gpsimd.collective_compute(
                "AllToAll", OP.bypass, replica_groups=GROUPS,
                ins=[srct.bitcast(F32)], outs=[dst.bitcast(F32)])

    with tile.TileContext(nc) as tc:
        with tc.tile_pool(name="const", bufs=1) as cpool, \
             tc.tile_pool(name="dram", bufs=1, space="DRAM") as dr:

            a2a_in_m = dr.tile([NCORES, B, SHB], F8, name="a2a_in_m")
            a2a_out_m = dr.tile([NCORES, B, SHB], F8, name="a2a_out_m")
            a2o_in_m = dr.tile([NCORES, B, 128, TC], F8, name="a2o_in_m")
            a2o_out_m = dr.tile([NCORES, B, 128, TC], F8, name="a2o_out_m")
            a2a_in = [a2a_in_m[:, b, :] for b in range(B)]
            a2a_out = [a2a_out_m[:, b, :] for b in range(B)]
            a2o_in = [a2o_in_m[:, b, :, :] for b in range(B)]
            a2o_out = [a2o_out_m[:, b, :, :] for b in range(B)]

            for _it in range(niters):
                # ========== P1: x load + transpose (DMA priority) ==========
                xT, xT_free = tc.tile([128, NDT, TPC], F32R, name="xT")
                xT8, xT8_free = tc.tile([128, NDT, TPC], F8, name="xT8")
                ident = cpool.tile([128, 128], F32)
                make_identity(nc, ident[:])
                with tc.tile_pool(name="p1", bufs=2) as p1, \
                     tc.tile_pool(name="pst", bufs=4, space="PSUM") as pst:
                    for tt in range(TPC // 128):
                        x_nat = p1.tile([128, D], F32, name="x_nat")
                        nc.sync.dma_start(x_nat[:], x_in[128 * tt:128 * (tt + 1), :])
                        for dt in range(NDT):
                            ps_t = pst.tile([128, 128], F32, name="ps_t")
                            nc.tensor.transpose(
                                ps_t[:], x_nat[:, 128 * dt:128 * (dt + 1)], ident[:])
                            nc.vector.tensor_copy(
                                xT[:, dt, 128 * tt:128 * (tt + 1)], ps_t[:])
                    # fp8 cast per batch half (feeds QKV DoubleRow matmuls)
                    for beta in range(B):
                        c0 = TC * beta
                        nc.scalar.activation(xT8[:, :, c0:c0 + TC],
                                             xT[:, :, c0:c0 + TC], AF.Copy)

                mark("P1_xT")
                # ========== P2: QKV projections (fp8 DoubleRow) ==========
                qt8, qt_free = tc.tile([128, NDT, TPC], F8, name="qt8")
                kt8, kt_free = tc.tile([128, NDT, TPC], F8, name="kt8")
                vaug8, vaug_free = tc.tile([128, TPC // 128, H, 64], F8, name="vaug8")
                wq_sb, wq_free = tc.tile([128, 4, 2, D], F8, name="wq_sb")
                wk_sb, wk_free = tc.tile([128, 4, 2, D], F8, name="wk_sb")
                wv_sb, wv_free = tc.tile([128, 4, 2, D], F8, name="wv_sb")
                for w_sb, w_dram in ((wq_sb, wq8), (wk_sb, wk8), (wv_sb, wv8)):
                    nc.sync.dma_start(
                        w_sb[:],
                        w_dram[:, :].rearrange("p (m i n) -> p m i n", m=4, i=2))
                # wo8 + FFN weights early on the scalar queue (idle at start):
                # they stream in the background without blocking SP's
                # latency-critical pack/unpack DMAs.
                with tc.tile_pool(name="w1s", bufs=3, side="right") as w1pool, \
                     tc.tile_pool(name="w2s", bufs=3, side="right") as w2pool:
                    wo_sb, wo_free = tc.tile([128, 4, 2, D], F8, name="wo_sb",
                                             side="right")
                    nc.sync.dma_start(
                        wo_sb[:],
                        wo8[:, :].rearrange("p (m i n) -> p m i n", m=4, i=2))
                    w1_tiles = {}
                    w2_tiles = {}

                    def _w1dma(fb):
                        w1_sb = w1pool.tile([128, NDT, 512], BF, name="w1_sb")
                        nc.sync.dma_start(
                            w1_sb[:],
                            w1h[:, 512 * fb:512 * (fb + 1)]
                                .rearrange("(c p) f -> p c f", p=128))
                        w1_tiles[fb] = w1_sb

                    def _w2dma(ftb):
                        w2_sb = w2pool.tile([128, 4, D], BF, name="w2_sb")
                        nc.sync.dma_start(
                            w2_sb[:],
                            w2h[512 * ftb:512 * (ftb + 1), :]
                                .rearrange("(f p) d -> p f d", p=128))
                        w2_tiles[ftb] = w2_sb

                    # constants & per-partition params
                    ones_f32 = cpool.tile([128, 128], F32)
                    nc.vector.memset(ones_f32[:], 1.0)
                    ones_c128 = cpool.tile([128, 1], F32R)
                    nc.vector.tensor_copy(ones_c128[:], ones_f32[:, 0:1])
                    ones_r128 = cpool.tile([1, 128], F32R)
                    nc.vector.tensor_copy(ones_r128[:], ones_f32[0:1, :])
                    # sliding causal mask: M[p, u] = 1 iff u - p >= 512
                    mask_f32 = cpool.tile([128, 1024], F32)
                    nc.gpsimd.memset(mask_f32[:], 1.0)
                    nc.gpsimd.affine_select(
                        out=mask_f32[:], in_=mask_f32[:],
                        compare_op=OP.is_ge, fill=0.0, base=-512,
                        pattern=[[1, 1024]], channel_multiplier=-1,
                    )
                    diag_mask8 = cpool.tile([128, 1024], F8)
                    nc.vector.tensor_copy(diag_mask8[:], mask_f32[:])

                    def load_pp(name, t, n):
                        sb = cpool.tile([128, n], F32, name=name)
                        nc.sync.dma_start(sb[:], t[:].rearrange("(a p) -> p a", p=128))
                        return sb

                    bq_sb = load_pp("bq_sb", bq, NDT)
                    bk_sb = load_pp("bk_sb", bk, NDT)
                    bo_sb = load_pp("bo_sb", bo, NDT)
                    b1_sb = load_pp("b1_sb", b1, NFT)
                    g1_sb = load_pp("g1_sb", ln1_s, NDT)
                    be1_sb = load_pp("be1_sb", ln1_b, NDT)

                    def load_bc(name, t):
                        sb = cpool.tile([128, D], F32, name=name)
                        nc.sync.dma_start(
                            sb[:], t[:].rearrange("(o d) -> o d", o=1)
                                .partition_broadcast(128)[:, 0, :])
                        return sb

                    b2_bc = load_bc("b2_bc", b2)
                    g2_bc = load_bc("g2_bc", ln2_s)
                    be2_bc = load_bc("be2_bc", ln2_b)
                    eps_sb = cpool.tile([1, 1], F32)
                    nc.vector.memset(eps_sb[:], float(EPS))
                    exb_sb = cpool.tile([128, 1], F32)
                    nc.vector.memset(exb_sb[:], float(EXB))
                    ones8 = cpool.tile([128, 2, 32], F8)
                    nc.vector.memset(ones8[:], 1.0)
                    eps_sb_p = cpool.tile([128, 1], F32)
                    nc.vector.memset(eps_sb_p[:], float(EPS))
                    bv_bc = cpool.tile([128, D], F32)
                    nc.sync.dma_start(
                        bv_bc[:],
                        bv[:].rearrange("(o d) -> o d", o=1)
                            .partition_broadcast(128)[:, 0, :])

                    with tc.tile_pool(name="psA", bufs=2, space="PSUM") as psA:
                        for beta in range(B):
                            c0 = TC * beta
                            for w_sb, dst8, b_sb in ((wq_sb, qt8, bq_sb),
                                                     (wk_sb, kt8, bk_sb)):
                                for dt in range(NDT):
                                    ps_q = psA.tile([128, TC], F32, name="ps_q")
                                    for m in range(4):
                                        nc.tensor.matmul(
                                            ps_q[:],
                                            w_sb[:, m, :, 128 * dt:128 * (dt + 1)],
                                            xT8[:, 2 * m:2 * m + 2, c0:c0 + TC],
                                            start=(m == 0), stop=(m == 3),
                                            perf_mode=DR)
                                    nc.vector.tensor_scalar(
                                        out=dst8[:, dt, c0:c0 + TC], in0=ps_q[:],
                                        scalar1=IWS, scalar2=b_sb[:, dt:dt + 1],
                                        op0=OP.mult, op1=OP.add)
                            for tt in range(2 * beta, 2 * beta + 2):
                                for hf in range(2):
                                    ps_v = psA.tile([128, 512], F32, name="ps_v")
                                    for m in range(4):
                                        nc.tensor.matmul(
                                            ps_v[:],
                                            xT8[:, 2 * m:2 * m + 2,
                                                128 * tt:128 * (tt + 1)],
                                            wv_sb[:, m, :, 512 * hf:512 * (hf + 1)],
                                            start=(m == 0), stop=(m == 3),
                                            perf_mode=DR)
                                    nc.vector.scalar_tensor_tensor(
                                        out=vaug8[:, tt, 8 * hf:8 * (hf + 1), :],
                                        in0=ps_v[:].rearrange("p (h e) -> p h e", h=8),
                                        scalar=IWS,
                                        in1=bv_bc[:, 512 * hf:512 * (hf + 1)].rearrange(
                                            "p (h e) -> p h e", h=8),
                                        op0=OP.mult, op1=OP.add)

                            # pack + A2A for this batch (fp8 payloads).
                            # Q shard d = dt-tile d (features 128d..128d+127,
                            # interleaved pair order preserved for unpack).
                            nc.sync.dma_start(
                                a2a_in[beta][:, 0:QK_SHB]
                                    .rearrange("d (p t) -> p d t", p=128),
                                qt8[:, :, c0:c0 + TC])
                            nc.sync.dma_start(
                                a2a_in[beta][:, QK_SHB:2 * QK_SHB]
                                    .rearrange("d (p t) -> p d t", p=128),
                                kt8[:, :, c0:c0 + TC])
                            for tt in range(2 * beta, 2 * beta + 2):
                                ttl = tt - 2 * beta
                                for hh in range(2):
                                    off = 2 * QK_SHB + ttl * 16384 + hh * 8192
                                    nc.sync.dma_start(
                                        a2a_in[beta][:, off:off + 8192]
                                            .rearrange("d (p c) -> p d c", p=128),
                                        vaug8[:, tt, :, :].rearrange(
                                            "p (d hh) c -> p d hh c",
                                            hh=2)[:, :, hh, :])
                            if beta == B - 1:
                                a2a(a2a_out_m[:].opt(), a2a_in_m[:].opt())
                            mark(f"P2_qkv_b{beta}")

                    wv_free()
                    wk_free()
                    wq_free()
                    vaug_free()
                    kt_free()
                    qt_free()
                    xT8_free()

                    # first FFN weight chunks stream during the A2A/attention
                    # (dep-free; ordered after the latency-critical packs)
                    for fb in range(3):
                        _w1dma(fb)
                    for ftb in range(3):
                        _w2dma(ftb)

                    r1, r1_free = tc.tile([128, NDT, TPC], F32R, name="r1")

                    # fold out-proj bias into the residual: xT += bo
                    for dt in range(NDT):
                        nc.vector.tensor_scalar(
                            out=xT[:, dt, :], in0=xT[:, dt, :],
                            scalar1=bo_sb[:, dt:dt + 1], scalar2=None, op0=OP.add)

                    # ========== P3: attention (2 heads x 2 batches, fp8) =====
                    NW = S // 512  # 4 q-windows of 512
                    with tc.tile_pool(name="att_io", bufs=2) as aio, \
                         tc.tile_pool(name="exp", bufs=3) as epool, \
                         tc.tile_pool(name="stage", bufs=4) as spool, \
                         tc.tile_pool(name="ps_sc", bufs=2, space="PSUM") as ps_sc, \
                         tc.tile_pool(name="ps_pv", bufs=2, space="PSUM") as ps_pv, \
                         tc.tile_pool(name="ps_dn", bufs=1, space="PSUM") as ps_dn, \
                         tc.tile_pool(name="ps_bc", bufs=1, space="PSUM") as ps_bc:

                        for beta in range(B):
                            # q8/k8: [32, head j, pair i, token]; partition p
                            # holds hd dims {2p, 2p+1} of head j.
                            q8 = aio.tile([32, 2, 2, S], F8, name="q8")
                            k8 = aio.tile([32, 2, 2, S], F8, name="k8")
                            va8 = aio.tile([128, 2, S // 128, 64], F8, name="va8")
                            for j in range(2):
                                for i in range(2):
                                    nc.sync.dma_start(
                                        q8[:, j, i, :].rearrange(
                                            "p (s t) -> p s t", s=NCORES),
                                        a2a_out[beta][:, 16384 * j:16384 * (j + 1)]
                                            .rearrange("s (pp two t) -> pp two s t",
                                                       two=2, t=TC)[:, i, :, :])
                                    nc.sync.dma_start(
                                        k8[:, j, i, :].rearrange(
                                            "p (s t) -> p s t", s=NCORES),
                                        a2a_out[beta][:, QK_SHB + 16384 * j:
                                                      QK_SHB + 16384 * (j + 1)]
                                            .rearrange("s (pp two t) -> pp two s t",
                                                       two=2, t=TC)[:, i, :, :])
                            for j in range(2):
                                for ttl in range(2):
                                    off = 2 * QK_SHB + ttl * 16384 + j * 8192
                                    nc.sync.dma_start(
                                        va8[:, j, ttl::2, :],
                                        a2a_out[beta][:, off:off + 8192]
                                            .rearrange("s (p c) -> p s c", p=128))

                            for w in range(NW):
                                q0 = 512 * w
                                for j in range(2):  # my two heads
                                    ps_o = ps_pv.tile([64, 512], F32, name="ps_o")
                                    ps_d = ps_dn.tile([32, 512], F32, name="ps_d")
                                    npair = 2 * w + 2
                                    for pr in range(npair):
                                        rels = [2 * pr - 4 * w, 2 * pr + 1 - 4 * w]
                                        qlo = max(0, 128 * rels[0])
                                        ps_s = ps_sc.tile([128, 1024], F32,
                                                          name="ps_s")
                                        ex8 = epool.tile([128, 2, 512], F8,
                                                         name="ex8")
                                        for u in range(2):
                                            kt_i = 2 * pr + u
                                            nc.tensor.matmul(
                                                ps_s[:, 512 * u + qlo:512 * (u + 1)],
                                                k8[:, j, :,
                                                   128 * kt_i:128 * (kt_i + 1)],
                                                q8[:, j, :, q0 + qlo:q0 + 512],
                                                start=True, stop=True,
                                                perf_mode=DR)
                                        if rels[1] < 0:
                                            # both tiles fully visible: wide exp
                                            nc.scalar.activation(
                                                ex8[:, :, :], ps_s[:],
                                                AF.Exp, scale=float(SCALE),
                                                bias=exb_sb[:])
                                        else:
                                            for u in range(2):
                                                nc.scalar.activation(
                                                    ex8[:, u, qlo:512],
                                                    ps_s[:, 512 * u + qlo:
                                                         512 * (u + 1)],
                                                    AF.Exp, scale=float(SCALE),
                                                    bias=exb_sb[:])
                                            for u in range(2):
                                                if rels[u] < 0:
                                                    continue
                                                moff = 512 - 128 * rels[u] + qlo
                                                nc.vector.tensor_tensor(
                                                    ex8[:, u, qlo:512],
                                                    ex8[:, u, qlo:512],
                                                    diag_mask8[:, moff:
                                                               moff + 512 - qlo],
                                                    op=OP.mult)
                                        nc.tensor.matmul(
                                            ps_o[:, qlo:512],
                                            va8[:, j, 2 * pr:2 * pr + 2, :],
                                            ex8[:, :, qlo:512],
                                            start=(pr == 0),
                                            stop=(pr == npair - 1),
                                            perf_mode=DR)
                                        nc.tensor.matmul(
                                            ps_d[:, qlo:512],
                                            ones8[:],
                                            ex8[:, :, qlo:512],
                                            start=(pr == 0),
                                            stop=(pr == npair - 1),
                                            perf_mode=DR)
                                    # normalize by ones-row denominator
                                    recip = spool.tile([1, 512], F32R, name="recip")
                                    with nc.allow_low_precision(
                                            reason="fp32r rounding of denom"):
                                        nc.vector.reciprocal(recip[:],
                                                             ps_d[0:1, :])
                                    ps_b = ps_bc.tile([64, 512], F32, name="ps_b")
                                    nc.tensor.matmul(ps_b[:], ones_r128[:, 0:64],
                                                     recip[:], start=True,
                                                     stop=True)
                                    rb = spool.tile([64, 512], F32, name="rb")
                                    nc.vector.tensor_copy(rb[:], ps_b[:])
                                    stg8 = spool.tile([64, 2, TC], F8, name="stg8")
                                    nc.vector.tensor_tensor(
                                        stg8[:].rearrange("p i t -> p (i t)"),
                                        ps_o[:], rb[:], op=OP.mult)
                                    r0 = 64 * j
                                    nc.sync.dma_start(
                                        a2o_in[beta][2 * w:2 * w + 2,
                                                     r0:r0 + 64, :]
                                            .rearrange("h p t -> p h t"),
                                        stg8[:])
                            if beta == B - 1:
                                a2a(a2o_out_m[:].opt(), a2o_in_m[:].opt())
                            mark(f"P3_attn_b{beta}")

                    # ========== P4: out-proj (fp8 DoubleRow) ==========
                    attn8, attn_free = tc.tile([128, NDT, TPC], F8, name="attn8")
                    with tc.tile_pool(name="psB", bufs=4, space="PSUM") as psB:
                        for beta in range(B):
                            c0 = TC * beta
                            nc.sync.dma_start(
                                attn8[:, :, c0:c0 + TC],
                                a2o_out[beta][:, :, :].rearrange("s p t -> p s t"))
                            for dt in range(NDT):
                                ps_po = psB.tile([128, TC], F32, name="ps_po")
                                for m in range(4):
                                    nc.tensor.matmul(
                                        ps_po[:],
                                        wo_sb[:, m, :, 128 * dt:128 * (dt + 1)],
                                        attn8[:, 2 * m:2 * m + 2, c0:c0 + TC],
                                        start=(m == 0), stop=(m == 3),
                                        perf_mode=DR)
                                nc.vector.scalar_tensor_tensor(
                                    out=r1[:, dt, c0:c0 + TC], in0=ps_po[:],
                                    scalar=IWS, in1=xT[:, dt, c0:c0 + TC],
                                    op0=OP.mult, op1=OP.add)
                    mark("P4_oproj")
                    attn_free()
                    wo_free()

                    # ========== P5: LN1 ==========
                    ln1, ln1_free = tc.tile([128, NDT, TPC], F32R, name="ln1",
                                            side="right")
                    _layernorm_T(nc, tc, r1, ln1, g1_sb, be1_sb, ones_c128,
                                 ones_r128, eps_sb)
                    mark("P5_ln1")
                    r1_free()
                    xT_free()

                    # bf16 cast for FFN1 + natural layout (+b2) for residual
                    ln1nb, ln1nb_free = tc.tile([128, TPC // 128, D], F32,
                                                name="ln1nb")
                    ln1h, ln1h_free = tc.tile([128, NDT, TPC], BF, name="ln1h")
                    nc.scalar.activation(ln1h[:, :, :], ln1[:, :, :], AF.Copy)
                    with tc.tile_pool(name="pstn", bufs=4, space="PSUM") as pstn:
                        for tt in range(TPC // 128):
                            for dt in range(NDT):
                                ps_tn = pstn.tile([128, 128], F32, name="ps_tn")
                                nc.tensor.transpose(
                                    ps_tn[:],
                                    ln1[:, dt, 128 * tt:128 * (tt + 1)].bitcast(F32),
                                    ident[:])
                                nc.vector.tensor_tensor(
                                    ln1nb[:, tt, 128 * dt:128 * (dt + 1)],
                                    ps_tn[:], b2_bc[:, 128 * dt:128 * (dt + 1)],
                                    op=OP.add)

                    # ========== P6: FFN1 (bf16) ==========
                    gT, gT_free = tc.tile([128, NFT, TPC], BF, name="gT",
                                          side="right")
                    with tc.tile_pool(name="psC", bufs=2, space="PSUM") as psC:
                        for fb in range(NFT // 4):
                            if fb + 3 < NFT // 4:
                                _w1dma(fb + 3)
                            w1_sb = w1_tiles[fb]
                            for fc in range(4):
                                ft = 4 * fb + fc
                                ps_h = psC.tile([128, TPC], F32, name="ps_h")
                                for ct in range(NDT):
                                    nc.tensor.matmul(
                                        ps_h[:],
                                        w1_sb[:, ct, 128 * fc:128 * (fc + 1)],
                                        ln1h[:, ct, :],
                                        start=(ct == 0), stop=(ct == NDT - 1))
                                nc.scalar.activation(gT[:, ft, :], ps_h[:],
                                                     GELU_F,
                                                     bias=b1_sb[:, ft:ft + 1])
                    mark("P6_ffn1")
                    ln1h_free()

                    # ========== P7: FFN2 (bf16) + residual + LN2 ==========
                    with tc.tile_pool(name="psD", bufs=1, space="PSUM") as psD:
                        ps_y = [psD.tile([128, 512], F32, name=f"ps_y{i}")
                                for i in range(8)]
                        for ftb in range(NFT // 4):
                            if ftb + 3 < NFT // 4:
                                _w2dma(ftb + 3)
                            w2_sb = w2_tiles[ftb]
                            for fl in range(4):
                                ft = 4 * ftb + fl
                                for tt in range(TPC // 128):
                                    for dh in range(2):
                                        nc.tensor.matmul(
                                            ps_y[2 * tt + dh][:],
                                            gT[:, ft, 128 * tt:128 * (tt + 1)],
                                            w2_sb[:, fl, 512 * dh:512 * (dh + 1)],
                                            start=(ft == 0),
                                            stop=(ft == NFT - 1))
                        # residual + LN2 (natural, per token tile) + store
                        with tc.tile_pool(name="lnn", bufs=2) as lnn, \
                             tc.tile_pool(name="lnsc", bufs=2) as lnsc:
                            for tt in range(TPC // 128):
                                r2n = lnn.tile([128, D], F32, name="r2n")
                                for dh in range(2):
                                    nc.vector.tensor_tensor(
                                        r2n[:, 512 * dh:512 * (dh + 1)],
                                        ps_y[2 * tt + dh][:],
                                        ln1nb[:, tt, 512 * dh:512 * (dh + 1)],
                                        op=OP.add)
                                ssum = lnsc.tile([128, 1], F32, name="ssum")
                                nc.vector.tensor_reduce(ssum[:], r2n[:],
                                                        axis=mybir.AxisListType.X,
                                                        op=OP.add)
                                sqs = lnsc.tile([128, D], F32, name="sqs")
                                s2 = lnsc.tile([128, 1], F32, name="s2")
                                nc.scalar.activation(sqs[:], r2n[:], AF.Square)
                                nc.vector.tensor_reduce(
                                    s2[:], sqs[:],
                                    axis=mybir.AxisListType.X, op=OP.add)
                                m = lnsc.tile([128, 1], F32, name="m")
                                nc.vector.tensor_scalar(out=m[:], in0=ssum[:],
                                                        scalar1=1.0 / D,
                                                        scalar2=None, op0=OP.mult)
                                e2 = lnsc.tile([128, 1], F32, name="e2")
                                nc.vector.tensor_scalar(out=e2[:], in0=s2[:],
                                                        scalar1=1.0 / D,
                                                        scalar2=None, op0=OP.mult)
                                msq = lnsc.tile([128, 1], F32, name="msq")
                                nc.vector.tensor_tensor(msq[:], m[:], m[:],
                                                        op=OP.mult)
                                var = lnsc.tile([128, 1], F32, name="var")
                                nc.vector.tensor_tensor(var[:], e2[:], msq[:],
                                                        op=OP.subtract)
                                std = lnsc.tile([128, 1], F32, name="std")
                                nc.scalar.activation(std[:], var[:], AF.Sqrt,
                                                     bias=eps_sb_p[:])
                                rstd = lnsc.tile([128, 1], F32, name="rstd")
                                nc.vector.reciprocal(rstd[:], std[:])
                                t_n = lnsc.tile([128, D], F32, name="t_n")
                                nc.vector.tensor_scalar(
                                    out=t_n[:], in0=r2n[:], scalar1=m[:],
                                    scalar2=rstd[:], op0=OP.subtract, op1=OP.mult)
                                t_g = lnsc.tile([128, D], F32, name="t_g")
                                nc.vector.tensor_tensor(t_g[:], t_n[:], g2_bc[:],
                                                        op=OP.mult)
                                o_n = lnn.tile([128, D], F32, name="o_n")
                                nc.vector.tensor_tensor(o_n[:], t_g[:], be2_bc[:],
                                                        op=OP.add)
                                nc.sync.dma_start(
                                    y_out[128 * tt:128 * (tt + 1), :], o_n[:])
                    mark("P7_ffn2")
                    gT_free()
                    ln1_free()
                    ln1nb_free()
                    mark("P9_out")

    nc.finalize()
    return nc


def _pack_dr(w):
    """Pack [K, N] weight into fp8 DoubleRow layout [128, (K//256)*2*N] x WS."""
    K, N = w.shape
    a = (np.asarray(w, np.float32) * WS).reshape(K // 256, 2, 128, N)
    a = a.transpose(2, 0, 1, 3).reshape(128, -1)
    return np.ascontiguousarray(a.astype(ml_dtypes.float8_e4m3))


def _get_nc():
    global _CACHED_NC
    if _CACHED_NC is None:
        _CACHED_NC = build_nc()
    return _CACHED_NC


def kernel(x, attention_mask, wq, bq, wk, bk, wv, bv, wo, bo,
           ln1_scale, ln1_bias, w1, b1, w2, b2, ln2_scale, ln2_bias):
    x = np.ascontiguousarray(np.asarray(x, dtype=np.float32))
    shared = {
        "wq8": _pack_dr(np.asarray(wq, np.float32)),
        "wk8": _pack_dr(np.asarray(wk, np.float32)),
        "wv8": _pack_dr(np.asarray(wv, np.float32)),
        "wo8": _pack_dr(np.asarray(wo, np.float32)),
        "w1h": np.ascontiguousarray(
            np.asarray(w1, np.float32).astype(ml_dtypes.bfloat16)),
        "w2h": np.ascontiguousarray(
            np.asarray(w2, np.float32).astype(ml_dtypes.bfloat16)),
        "bq": np.asarray(bq, np.float32), "bk": np.asarray(bk, np.float32),
        "bv": np.asarray(bv, np.float32), "bo": np.asarray(bo, np.float32),
        "b1": np.asarray(b1, np.float32), "b2": np.asarray(b2, np.float32),
        "ln1_s": np.asarray(ln1_scale, np.float32),
        "ln1_b": np.asarray(ln1_bias, np.float32),
        "ln2_s": np.asarray(ln2_scale, np.float32),
        "ln2_b": np.asarray(ln2_bias, np.float32),
    }
    in_maps = []
    for c in range(NCORES):
        x_own = np.concatenate(
            [x[0, TC * c:TC * (c + 1)], x[1, TC * c:TC * (c + 1)]], axis=0)
        in_maps.append({"x_own": np.ascontiguousarray(x_own), **shared})

    nc = _get_nc()
    res = run_bass_kernel_spmd(nc, in_maps, core_ids=list(range(NCORES)))
    out = np.empty((B, S, D), np.float32)
    for c in range(NCORES):
        y = res.results[c]["y"]
        out[0, TC * c:TC * (c + 1)] = y[0:TC]
        out[1, TC * c:TC * (c + 1)] = y[TC:TPC]
    return out
